# revision 1
# baseline (speedup 1.0000x reference)
"""Trainium2 Bass kernel for nn_DepthWiseSepConv (depthwise 5x5 + BN+hardswish
+ pointwise 1x1 + squeeze-excite gating + BN), data-parallel over batch on
8 NeuronCores.

Self-contained: hardcodes all shapes from the problem spec.

Per-core layout strategy (B_loc = 8 images per core):
  - Depthwise conv: partitions = (4 channels x 28 rows of H). For each of the
    5 kernel columns dx, one matmul with a host-built block-diagonal Toeplitz
    matrix (contracting h_in -> h_out) against x shifted by dx along W (zero
    padded in SBUF). The 5 matmuls accumulate in PSUM.
  - BN1 + hardswish fused: ACT relu(psum*s1 + t1+3), then
    act = (a-3) * min(a/6, 1).
  - Two TensorE transpose stages to reach channel-major [c, (b,h,w)] layout
    for the pointwise conv.
  - SE: DVE free-dim reduce for the mean, two small matmuls, hardswish.
  - Pointwise conv: [120c x 120o] matmul tiles, N=392 (half an image),
    epilogue fuses +pw_b, *g (SE gate), BN2.
"""

import sys

sys.path.insert(0, "/opt/trn_rl_repo")

import numpy as np
import ml_dtypes

import concourse.bass as bass
import concourse.mybir as mybir
import concourse.tile as tile
from concourse import bacc
from concourse.bass_utils import run_bass_kernel_spmd
from concourse.masks import make_identity

# ---------------------------------------------------------------- constants
N_CORES = 8
B, C, H, W = 64, 240, 28, 28
NB = B // N_CORES          # images per core
KK = 5                      # depthwise kernel size
G = C // 4                  # channel groups of 4 -> 60
R = 60                      # SE reduction dim
Cout = 240
HW = H * W                  # 784
EPS = 1e-5
WP = 36                     # padded W in SBUF x tiles (w in [-2, 34))
PIX = NB * HW               # 6272 pixels per core

CFG = {
    # dtype of DW + PW matmul operands: "float32" | "float32r" | "bfloat16"
    "mm_dt": "float32r",
    # dtype of activation storage / transposes: "float32" | "bfloat16"
    "act_dt": "float32",
    # DW psum free width: 28 (exact) or 32 (padded, helps float32r)
    "wout": 32,
    # pack DW Toeplitz as 4x[32,32] tile_position blocks (3x less weight DMA)
    "packed": False,
    # DRAM storage dtype of toep; float16 halves DMA, cast to f32 in flight
    "toep_store": "float16",
    # transfer only the 4 diagonal 28x28 blocks of each Toeplitz (4x less
    # DMA); persistent pre-zeroed SBUF buffers, manual 3-way rotation
    "toep_compact": False,
    # debug: emit only a prefix of the phases ("a"|"ab"|"abc"|"")
    "stop_after": "",
    # x DMA batching: groups loaded per DMA (1 or 2)
    "xbatch": 1,
    # rotation depth for x/toep persistent buffers
    "nrot": 4,
    # DW psum pool depth
    "dwbufs": 4,
}

_DT = {
    "float32": mybir.dt.float32,
    "float32r": mybir.dt.float32r,
    "bfloat16": mybir.dt.bfloat16,
}
_NPDT = {
    "float32": np.float32,
    "float32r": np.float32,
    "bfloat16": ml_dtypes.bfloat16,
}


def _f32v(ap):
    """View a float32r AP as plain float32 (for non-matmul readers)."""
    if ap.dtype == mybir.dt.float32r:
        return ap.bitcast(mybir.dt.float32)
    return ap


# ---------------------------------------------------------------- builder
_BUILD_CACHE = {}


def build_nc(cfg_key=None):
    cfg = dict(CFG)
    if cfg_key is not None:
        cfg.update(cfg_key)
    key = tuple(sorted(cfg.items()))
    if key in _BUILD_CACHE:
        return _BUILD_CACHE[key]

    mm_dt = _DT[cfg["mm_dt"]]
    act_dt = _DT[cfg["act_dt"]]
    WOUT = cfg["wout"]
    dw_r = cfg["mm_dt"] == "float32r"
    pw_r = dw_r and cfg["act_dt"] == "float32"
    # dtype of the PW matmul operands (weights + transposed activations)
    pw_dt = mybir.dt.float32r if pw_r else act_dt

    nc = bacc.Bacc("TRN2", target_bir_lowering=False, debug=False,
                   num_devices=N_CORES)

    packed = cfg["packed"]
    DWP = 128 if packed else 112     # DW partition count
    HB = 32 if packed else H         # per-channel partition block
    toep_st = mm_dt
    if cfg["toep_store"] == "float16" and cfg["mm_dt"] != "bfloat16":
        toep_st = mybir.dt.float16

    f32 = mybir.dt.float32
    x_dram_dt = f32 if cfg["mm_dt"] == "bfloat16" else mm_dt
    x_p = nc.declare_dram_parameter("x", [NB, C, H, W], x_dram_dt,
                                    isOutput=False)
    if packed:
        toep_p = nc.declare_dram_parameter("toep", [G, 4, 32, KK, 32], toep_st,
                                           isOutput=False)
    elif cfg["toep_compact"]:
        toep_p = nc.declare_dram_parameter("toep", [G, 4, H, KK, H], toep_st,
                                           isOutput=False)
    else:
        toep_p = nc.declare_dram_parameter("toep", [G, 112, KK, 112], toep_st,
                                           isOutput=False)
    bn1s_p = nc.declare_dram_parameter("bn1s", [DWP, G], f32, isOutput=False)
    bn1b_p = nc.declare_dram_parameter("bn1b", [DWP, G], f32, isOutput=False)
    pwl_p = nc.declare_dram_parameter("pwl", [2, 120, 2, 120], pw_dt,
                                      isOutput=False)
    se1l_p = nc.declare_dram_parameter("se1l", [2, 120, R], f32, isOutput=False)
    se1b_p = nc.declare_dram_parameter("se1b", [R, 1], f32, isOutput=False)
    se2l_p = nc.declare_dram_parameter("se2l", [R, 2, 120], f32, isOutput=False)
    se2b3_p = nc.declare_dram_parameter("se2b3", [120, 2], f32, isOutput=False)
    bn2s_p = nc.declare_dram_parameter("bn2s", [120, 2], f32, isOutput=False)
    bn2sb_p = nc.declare_dram_parameter("bn2sb", [120, 2], f32, isOutput=False)
    bn2t_p = nc.declare_dram_parameter("bn2t", [120, 2], f32, isOutput=False)
    # zero-fill source (walrus rejects Memset on float32r tiles)
    zeros_p = nc.declare_dram_parameter("zeros", [128, 640], mm_dt,
                                        isOutput=False)
    y_p = nc.declare_dram_parameter("y", [NB, Cout, H, W], f32, isOutput=True)

    AL = mybir.AluOpType

    with tile.TileContext(nc) as tc:
        cst = tc.alloc_tile_pool(name="cst", bufs=1)
        pers = tc.alloc_tile_pool(name="pers", bufs=1)

        # ---- constants in SBUF
        bn1s_sb = cst.tile([DWP, G], f32)
        nc.sync.dma_start(bn1s_sb[:], bn1s_p[:])
        bn1b_sb = cst.tile([DWP, G], f32)
        nc.sync.dma_start(bn1b_sb[:], bn1b_p[:])
        pwl_sb = cst.tile([120, 2, 2, 120], pw_dt)  # [K=c, kc, mo, M=o]
        nc.sync.dma_start(pwl_sb[:], pwl_p[:].rearrange("kc k mo m -> k kc mo m"))
        se1l_sb = cst.tile([120, 2, R], f32)
        nc.sync.dma_start(se1l_sb[:], se1l_p[:].rearrange("kc k r -> k kc r"))
        se1b_sb = cst.tile([R, 1], f32)
        nc.sync.dma_start(se1b_sb[:], se1b_p[:])
        se2l_sb = cst.tile([R, 2, 120], f32)
        nc.sync.dma_start(se2l_sb[:], se2l_p[:])
        se2b3_sb = cst.tile([120, 2], f32)
        nc.sync.dma_start(se2b3_sb[:], se2b3_p[:])
        bn2s_sb = cst.tile([120, 2], f32)
        nc.sync.dma_start(bn2s_sb[:], bn2s_p[:])
        bn2sb_sb = cst.tile([120, 2], f32)
        nc.sync.dma_start(bn2sb_sb[:], bn2sb_p[:])
        bn2t_sb = cst.tile([120, 2], f32)
        nc.sync.dma_start(bn2t_sb[:], bn2t_p[:])

        ident = cst.tile([128, 128], act_dt)
        make_identity(nc, ident[:])

        # persistent activation buffers
        # ActT[ch]: [(b4,w28)=112, q, (g_local, c4, h) = 30*112]
        ActT = [pers.tile([112, 2, 30 * 112], act_dt, name=f"actt_{ch}")
                for ch in range(2)]
        # PWrhs[ch]: [c=120, (b, h, w) = 6272]
        PWrhs = [pers.tile([120, PIX], pw_dt, name=f"pwrhs_{ch}")
                 for ch in range(2)]
        g_sb = [pers.tile([120, NB], f32, name=f"gate_{mo}") for mo in range(2)]

        # x rearranged for DW rhs: dims (c4, h, g, b, w)
        x_r = x_p[:].rearrange("b (g c) h w -> c h g b w", c=4)
        # merged (c h) partition form for the unpacked single-DMA load
        x_rm = x_p[:].rearrange("b (g c) h w -> (c h) g b w", c=4)

        # persistent DW input buffers, manual 3-way rotation: zero padding is
        # written once, per-group DMAs only overwrite the payload regions
        NROT = cfg["nrot"]
        XB = cfg["xbatch"]
        x_bufs = [pers.tile([DWP, XB, NB, WP], mm_dt, name=f"x_rot{i}")
                  for i in range(NROT)]
        zx = zeros_p[:, :XB * NB * WP].rearrange(
            "p (xb nb wp) -> p xb nb wp", xb=XB, nb=NB)
        for xb in x_bufs:
            nc.sync.dma_start(xb[:], zx[:DWP])
        toep_bufs = None
        if cfg["toep_compact"] and not packed:
            toep_bufs = [pers.tile([112, KK, 112], mm_dt, name=f"tp_rot{i}")
                         for i in range(NROT)]
            zt = zeros_p[:, :KK * 112].rearrange("p (k m) -> p k m", k=KK)
            for tb in toep_bufs:
                nc.sync.dma_start(tb[:], zt[:112])

        # ================= Phase A: depthwise + BN1 + hardswish + T1
        with tc.tile_pool(name="pa", bufs=3) as pa, \
             tc.tile_pool(name="pa2", bufs=4) as pa2, \
             tc.tile_pool(name="dwps", bufs=cfg["dwbufs"], space="PSUM") as dwps, \
             tc.tile_pool(name="t1ps", bufs=3, space="PSUM") as t1ps:
            for g in range(G):
                gb, gi = divmod(g, XB)
                x_gb = x_bufs[gb % NROT]
                x_g = x_gb[:, gi]
                x_dma = (nc.gpsimd.dma_start
                         if cfg["mm_dt"] == "bfloat16" else nc.sync.dma_start)
                if gi == 0:
                    if packed:
                        # SBUF APs have a single partition dim -> one DMA
                        # per 32-row channel block
                        for ci in range(4):
                            x_dma(x_gb[32 * ci:32 * ci + H, 0, :, 2:2 + W],
                                  x_r[ci, :, g])
                    elif XB == 1:
                        x_dma(x_gb[:, 0, :, 2:2 + W], x_rm[:, g])
                    else:
                        x_dma(x_gb[:, :, :, 2:2 + W],
                              x_rm[:, g:g + XB])
                toep_dma = (nc.gpsimd.dma_start if toep_st != mm_dt
                            else nc.sync.dma_start)
                if packed:
                    toep_g = pa.tile([128, KK, 32], mm_dt, tag="toep_g")
                    toep_dma(toep_g[:],
                             toep_p[g].rearrange("c e dx m -> (c e) dx m"))
                elif cfg["toep_compact"]:
                    toep_g = toep_bufs[g % NROT]
                    for ci in range(4):
                        toep_dma(
                            toep_g[H * ci:H * ci + H, :, H * ci:H * ci + H],
                            toep_p[g, ci])
                else:
                    toep_g = pa.tile([112, KK, 112], mm_dt, tag="toep_g")
                    toep_dma(toep_g[:], toep_p[g])

                ps = dwps.tile([DWP, NB, WOUT], f32, tag="dw")
                for dx in range(KK):
                    if packed:
                        # one accumulation group for the whole psum region:
                        # start clears has_written bank-wide; per-element
                        # has_written handles first-write-overwrite for the
                        # other 32-row blocks
                        for ci in range(4):
                            nc.tensor.matmul(
                                ps[32 * ci:32 * ci + 32],
                                toep_g[32 * ci:32 * ci + 32, dx, :],
                                x_g[32 * ci:32 * ci + 32, :, dx:dx + WOUT],
                                start=(dx == 0 and ci == 0),
                                stop=(dx == KK - 1 and ci == 3),
                                tile_position=(32 * ci, 32 * ci),
                            )
                    else:
                        nc.tensor.matmul(
                            ps[:],
                            toep_g[:, dx, :],
                            x_g[:, :, dx:dx + WOUT],
                            start=(dx == 0),
                            stop=(dx == KK - 1),
                        )

                a_g = pa2.tile([DWP, NB, WOUT], f32, tag="a_g")
                nc.scalar.activation(a_g[:], ps[:],
                                     mybir.ActivationFunctionType.Relu,
                                     bias=bn1b_sb[:, g:g + 1],
                                     scale=bn1s_sb[:, g:g + 1])
                a_v = a_g[:, :, 0:W]
                m_g = pa2.tile([DWP, NB, W], f32, tag="m_g")
                nc.gpsimd.tensor_scalar(m_g[:], a_v, 1.0 / 6.0, 1.0,
                                        AL.mult, AL.min)
                act_g = pa2.tile([DWP, NB, W], act_dt, tag="act_g")
                nc.vector.scalar_tensor_tensor(act_g[:], a_v, 3.0, m_g[:],
                                               AL.subtract, AL.mult)

                ch, gl = (0, g) if g < 30 else (1, g - 30)
                tp = t1ps.tile([112, 2, DWP], act_dt, tag="t1")
                for q in range(2):
                    nc.tensor.transpose(tp[:, q, :],
                                        act_g[:, 4 * q:4 * q + 4, :],
                                        ident[:DWP, :DWP])
                # select real (c4, h) columns out of each DWP block
                tp_sel = tp[:].rearrange("p q (c e) -> p q c e", c=4)[
                    :, :, :, 0:H]
                nc.scalar.copy(
                    ActT[ch][:, :, gl * 112:(gl + 1) * 112].rearrange(
                        "p q (c e) -> p q c e", c=4),
                    tp_sel)

        # ================= Phase B: T2 -> channel-major PWrhs
        with tc.tile_pool(name="t2ps", bufs=6, space="PSUM") as t2ps:
            for ch in range(2):
                for q in range(2):
                    src4 = ActT[ch][:].rearrange(
                        "p q (gl c e) -> p q gl c e", gl=30, c=4)
                    dst4 = PWrhs[ch][:].rearrange(
                        "p (b hh w) -> p b hh w", b=NB, hh=H)
                    for h0 in range(0, H, 4):
                        tp = t2ps.tile([120, 4, 112], act_dt, tag="t2")
                        for hi in range(4):
                            # 120 cols: (g_local str 112) x (c4 str 28), off h
                            nc.tensor.transpose(tp[:, hi, :],
                                                src4[:, q, :, :, h0 + hi],
                                                ident[:112, :112])
                        nc.vector.tensor_copy(
                            dst4[:, 4 * q:4 * q + 4, h0:h0 + 4, :],
                            tp[:].rearrange("p hh (b w) -> p b hh w", b=4))

        # ================= Phase C: squeeze-excite
        with tc.tile_pool(name="se", bufs=1) as sep, \
             tc.tile_pool(name="seps", bufs=2, space="PSUM") as seps:
            s_sb = [sep.tile([120, NB], f32, name=f"s_{ch}") for ch in range(2)]
            for ch in range(2):
                nc.vector.tensor_reduce(
                    s_sb[ch][:],
                    _f32v(PWrhs[ch][:]).rearrange("p (b f) -> p b f", b=NB),
                    mybir.AxisListType.X, AL.add)
            ps1 = seps.tile([R, NB], f32, tag="se1")
            for ch in range(2):
                nc.tensor.matmul(ps1[:], se1l_sb[:, ch, :], s_sb[ch][:],
                                 start=(ch == 0), stop=(ch == 1))
            h1 = sep.tile([R, NB], f32)
            nc.scalar.activation(h1[:], ps1[:],
                                 mybir.ActivationFunctionType.Relu,
                                 bias=se1b_sb[:, 0:1])
            for mo in range(2):
                ps2 = seps.tile([120, NB], f32, tag="se2")
                nc.tensor.matmul(ps2[:], se2l_sb[:, mo, :], h1[:],
                                 start=True, stop=True)
                a2 = sep.tile([120, NB], f32, name=f"a2_{mo}")
                nc.scalar.activation(a2[:], ps2[:],
                                     mybir.ActivationFunctionType.Relu,
                                     bias=se2b3_sb[:, mo:mo + 1])
                m2 = sep.tile([120, NB], f32, name=f"m2_{mo}")
                nc.vector.tensor_scalar(m2[:], a2[:], 1.0 / 6.0, 1.0,
                                        AL.mult, AL.min)
                nc.vector.scalar_tensor_tensor(g_sb[mo][:], a2[:], 3.0, m2[:],
                                               AL.subtract, AL.mult)

        # ================= Phase D: pointwise conv + gate + BN2 + output
        NT = 392  # half an image
        with tc.tile_pool(name="pd", bufs=6) as pd, \
             tc.tile_pool(name="pdps", bufs=4, space="PSUM") as pdps:
            for mo in range(2):
                for b in range(NB):
                    for nt in range(2):
                        off = b * HW + nt * NT
                        ps = pdps.tile([120, NT], f32, tag="pw")
                        for kc in range(2):
                            nc.tensor.matmul(
                                ps[:],
                                pwl_sb[:, kc, mo, :],
                                PWrhs[kc][:, off:off + NT],
                                start=(kc == 0), stop=(kc == 1))
                        e2 = pd.tile([120, NT], f32, tag="e2")
                        nc.scalar.activation(
                            e2[:], ps[:],
                            mybir.ActivationFunctionType.Identity,
                            bias=bn2sb_sb[:, mo:mo + 1],
                            scale=bn2s_sb[:, mo:mo + 1])
                        f_t = pd.tile([120, NT], f32, tag="f_t")
                        nc.vector.tensor_tensor(
                            f_t[:], e2[:],
                            g_sb[mo][:, b:b + 1].to_broadcast((120, NT)),
                            AL.mult)
                        o_t = pd.tile([120, NT], f32, tag="o_t")
                        nc.gpsimd.tensor_scalar(o_t[:], f_t[:],
                                                bn2t_sb[:, mo:mo + 1], None,
                                                AL.add)
                        y_ap = y_p[b, mo * 120:(mo + 1) * 120].rearrange(
                            "c h w -> c (h w)")[:, nt * NT:(nt + 1) * NT]
                        nc.sync.dma_start(y_ap, o_t[:])

        pers.release()
        cst.release()

    nc.compile()
    _BUILD_CACHE[key] = nc
    return nc


# ---------------------------------------------------------------- host prep
def prep_inputs(inputs, cfg_key=None):
    cfg = dict(CFG)
    if cfg_key is not None:
        cfg.update(cfg_key)
    mmnp = _NPDT[cfg["mm_dt"]]
    f32 = np.float32

    x = np.asarray(inputs["x"], f32)
    dw_w = np.asarray(inputs["dw_w"], f32)      # [C,1,5,5]
    dw_b = np.asarray(inputs["dw_b"], f32)
    bn1_g = np.asarray(inputs["bn1_g"], f32)
    bn1_b = np.asarray(inputs["bn1_b"], f32)
    bn1_m = np.asarray(inputs["bn1_m"], f32)
    bn1_v = np.asarray(inputs["bn1_v"], f32)
    pw_w = np.asarray(inputs["pw_w"], f32)      # [Cout, C]
    pw_b = np.asarray(inputs["pw_b"], f32)
    se_w1 = np.asarray(inputs["se_w1"], f32)    # [R, C]
    se_b1 = np.asarray(inputs["se_b1"], f32)
    se_w2 = np.asarray(inputs["se_w2"], f32)    # [Cout, R]
    se_b2 = np.asarray(inputs["se_b2"], f32)
    bn2_g = np.asarray(inputs["bn2_g"], f32)
    bn2_b = np.asarray(inputs["bn2_b"], f32)
    bn2_m = np.asarray(inputs["bn2_m"], f32)
    bn2_v = np.asarray(inputs["bn2_v"], f32)

    packed = cfg["packed"]
    HB = 32 if packed else H
    s1 = bn1_g / np.sqrt(bn1_v + EPS)
    t1 = s1 * (dw_b - bn1_m) + bn1_b

    def _pp(v):  # [C] -> [DWP, G] per-partition vector, zero-padded blocks
        a = np.zeros((G, 4, HB), f32)
        a[:, :, :H] = v.reshape(G, 4)[:, :, None]
        return np.ascontiguousarray(a.reshape(G, 4 * HB).T)

    bn1s = _pp(s1)
    bn1b = _pp(t1 + 3.0)

    # Toeplitz blockdiag: toep[g, ci*28+hin, dx, cj*28+hout]
    #   = dw_w[4g+ci, 0, hin-hout+2, dx] if ci==cj and |hin-hout|<=2
    hin = np.arange(H)[:, None]
    hout = np.arange(H)[None, :]
    D = hin - hout
    mask = np.abs(D) <= 2
    dyi = np.clip(D + 2, 0, 4)
    k = dw_w[:, 0]                                                # [C, 5, 5]
    # band[c, hin, hout, dx]
    band = np.where(mask[None, :, :, None], k[:, dyi, :], 0.0)    # [C,28,28,5]
    band_r = band.reshape(G, 4, H, H, KK)           # [g, ci, hin, hout, dx]
    if packed:
        # [G, 4, 32(hin), KK, 32(hout)] zero-padded per-channel blocks
        toep = np.zeros((G, 4, 32, KK, 32), f32)
        toep[:, :, :H, :, :H] = band_r.transpose(0, 1, 2, 4, 3)
    elif cfg["toep_compact"]:
        # [G, 4, hin, KK, hout] dense diagonal blocks only
        toep = np.ascontiguousarray(band_r.transpose(0, 1, 2, 4, 3))
    else:
        toep = np.zeros((G, 4, H, KK, 4, H), f32)
        for ci in range(4):
            # [g, hin, dx, hout]
            toep[:, ci, :, :, ci, :] = band_r[:, ci].transpose(0, 1, 3, 2)
        toep = toep.reshape(G, 112, KK, 112)
    if cfg["toep_store"] == "float16" and cfg["mm_dt"] != "bfloat16":
        toep = toep.astype(np.float16)
    else:
        toep = toep.astype(mmnp)

    actnp = _NPDT[cfg["act_dt"]]
    pwT = np.ascontiguousarray(pw_w.T)               # [C, Cout]
    pwl = np.zeros((2, 120, 2, 120), f32)
    for kc in range(2):
        for mo in range(2):
            pwl[kc, :, mo, :] = pwT[kc * 120:(kc + 1) * 120,
                                    mo * 120:(mo + 1) * 120]
    pwl = pwl.astype(actnp)

    se1l = np.ascontiguousarray((se_w1.T / HW).reshape(2, 120, R))
    se1b = se_b1.reshape(R, 1).copy()
    se2l = np.ascontiguousarray(se_w2.T.reshape(R, 2, 120))
    se2b3 = np.ascontiguousarray((se_b2 + 3.0).reshape(2, 120).T)
    s2 = bn2_g / np.sqrt(bn2_v + EPS)
    bn2s = np.ascontiguousarray(s2.reshape(2, 120).T)
    bn2sb = np.ascontiguousarray((s2 * pw_b).reshape(2, 120).T)
    bn2t = np.ascontiguousarray((bn2_b - bn2_m * s2).reshape(2, 120).T)

    shared = {
        "toep": toep, "bn1s": bn1s, "bn1b": bn1b, "pwl": pwl,
        "se1l": se1l.astype(f32), "se1b": se1b, "se2l": se2l.astype(f32),
        "se2b3": se2b3, "bn2s": bn2s, "bn2sb": bn2sb, "bn2t": bn2t,
        "zeros": np.zeros((128, 640), mmnp),
    }
    in_maps = []
    for i in range(N_CORES):
        m = dict(shared)
        m["x"] = np.ascontiguousarray(x[i * NB:(i + 1) * NB])
        in_maps.append(m)
    return in_maps


def kernel(**inputs):
    nc = build_nc()
    in_maps = prep_inputs(inputs)
    res = run_bass_kernel_spmd(nc, in_maps, list(range(N_CORES)))
    out = np.concatenate([res.results[i]["y"] for i in range(N_CORES)], axis=0)
    return out.astype(np.float32)



# revision 48
# speedup vs baseline: 2.6076x; 2.6076x over previous
"""Trainium2 Bass kernel for nn_DepthWiseSepConv (depthwise 5x5 + BN+hardswish
+ pointwise 1x1 + squeeze-excite gating + BN), data-parallel over batch on
8 NeuronCores.

Self-contained: hardcodes all shapes from the problem spec.

Per-core design (B_loc = 8 images per core), fp16 matmul path:

  - Depthwise conv as H-Toeplitz matmuls with *x as the stationary operand*:
    out[(b4,w28), (c4,hout)] accumulates over (ci, dx) with
    lhsT = x window [32 hin rows, (b4, w28)] at tile_position (32*ci, 0) and
    rhs = compact per-channel Toeplitz block [32, 28].  This needs only the
    COMPACT Toeplitz in SBUF (no block-diagonal expansion) and produces the
    output already transposed to (b,w)-major, eliminating one transpose pass.
  - BN1 is folded into the matmul: Toeplitz values are pre-scaled by the BN
    scale, and row 31 of each 32-row x block is constant 1.0 so that a
    bias row in the Toeplitz adds (t1 + 3) during accumulation.
  - hardswish(v) = (a-3)*clip(a,0,6)/6 with a = v+3: one Pool clip op and one
    DVE scalar_tensor_tensor writing fp16 into ActT (the /6 is folded into
    the pointwise weights and SE weights host-side).
  - T2: PE transposes ActT[(b4,w), (q,h,c')] slices into channel-major
    PWrhs[c', (q,h,b4,w)].  SE channel means ride along as tiny ones-matmuls
    accumulating in PSUM (exact f32 sums, nearly free on PE).
  - SE: two small fp32 matmuls; the SE2 bias rides a constant-1 row of h1.
    The gate is folded into per-(channel,batch) scale/bias tables so the
    whole PW epilogue (pw_b, gate, BN2) is a single op per tile.
  - Pointwise conv: [120x120] fp16 matmuls over half-images (N=392),
    epilogue alternates Act/DVE, output staged fp16 and upcast on host.
"""

import sys

sys.path.insert(0, "/opt/trn_rl_repo")

import numpy as np

import concourse.bass as bass
import concourse.mybir as mybir
import concourse.tile as tile
from concourse import bacc
from concourse.bass_utils import run_bass_kernel_spmd
from concourse.masks import make_identity

# ---------------------------------------------------------------- constants
N_CORES = 8
B, C, H, W = 64, 240, 28, 28
NB = B // N_CORES           # images per core
KK = 5                      # depthwise kernel size
G = C // 4                  # channel groups of 4 -> 60
R = 60                      # SE reduction dim
HWF = H * W                 # 784
EPS = 1e-5
WP = 32                     # padded W in SBUF (w in [-2, 30))

CFG = {
    "ld_step": 4,           # groups per x/toeplitz load DMA
    "dw_bufs": 4,           # DW psum pool depth (2 groups per bank)
    "m_bufs": 4,            # hardswish temp pool depth
    "t2_bufs": 3,           # T2 psum pool depth
    "pd_bufs": 4,           # PW psum pool depth
    "a1_early": 3,          # A(ch1) pairs emitted before B(ch0)
    "epi_dve": True,        # alternate PW epilogue between Act and DVE
    "y_f32": False,         # emit f32 output instead of fp16+host upcast
}

_DT16 = mybir.dt.float16
_F32 = mybir.dt.float32


# ---------------------------------------------------------------- builder
_BUILD_CACHE = {}


def build_nc(cfg_key=None):
    cfg = dict(CFG)
    if cfg_key is not None:
        cfg.update(cfg_key)
    key = tuple(sorted(cfg.items()))
    if key in _BUILD_CACHE:
        return _BUILD_CACHE[key]

    f16, f32 = _DT16, _F32
    AL = mybir.AluOpType
    AF = mybir.ActivationFunctionType
    ydt = f32 if cfg["y_f32"] else f16

    nc = bacc.Bacc("TRN2", target_bir_lowering=False, debug=False,
                   num_devices=N_CORES)

    # x layout [113, G, q2, w32, b4]: (w, b)-inner so a 28-w window over
    # 4 images merges to ONE contiguous free dim (matmul weights APs must
    # have a single free dimension)
    x_p = nc.declare_dram_parameter("x16", [113, G, 2, WP, 4], f16,
                                    isOutput=False)
    tc_p = nc.declare_dram_parameter("toepx", [113, G, KK, 112], f16,
                                     isOutput=False)
    pwl_p = nc.declare_dram_parameter("pwl", [120, 2, 2, 120], f16,
                                      isOutput=False)
    ones4_p = nc.declare_dram_parameter("ones4", [112, 4], f16,
                                        isOutput=False)
    se1l_p = nc.declare_dram_parameter("se1l", [120, 2, R], f32,
                                       isOutput=False)
    se1b_p = nc.declare_dram_parameter("se1b", [R, 1], f32, isOutput=False)
    se2l_p = nc.declare_dram_parameter("se2l", [61, 2, 120], f32,
                                       isOutput=False)
    s26_p = nc.declare_dram_parameter("s26", [120, 2], f32, isOutput=False)
    pb26_p = nc.declare_dram_parameter("pb26", [120, 2], f32, isOutput=False)
    t2c_p = nc.declare_dram_parameter("t2c", [120, 2], f32, isOutput=False)
    y_p = nc.declare_dram_parameter("y16", [NB, C, HWF], ydt, isOutput=True)

    with tile.TileContext(nc) as tc:
        cst = tc.alloc_tile_pool(name="cst", bufs=1)
        pers = tc.alloc_tile_pool(name="pers", bufs=1)

        # ---- x / toeplitz first, interleaved by group range, so phase A can
        # start as soon as the first chunk lands
        x_sb = pers.tile([113, G, 2, WP, 4], f16, name="x16")
        tc_sb = pers.tile([113, G, KK, 112], f16, name="toepx")
        step = cfg["ld_step"]
        sizes = [1, 1, 2, 2]
        while sum(sizes) < G:
            sizes.append(min(step, G - sum(sizes)))
        a = 0
        for sz in sizes:
            b_ = a + sz
            nc.sync.dma_start(tc_sb[:, a:b_], tc_p[:, a:b_])
            nc.sync.dma_start(x_sb[:, a:b_], x_p[:, a:b_])
            a = b_

        # ---- constants in SBUF
        ident = cst.tile([112, 112], f16)
        make_identity(nc, ident[:])
        ones4_sb = cst.tile([112, 4], f16)
        nc.sync.dma_start(ones4_sb[:], ones4_p[:])
        pwl_sb = cst.tile([120, 2, 2, 120], f16)
        nc.sync.dma_start(pwl_sb[:], pwl_p[:])
        se1l_sb = cst.tile([120, 2, R], f32)
        nc.sync.dma_start(se1l_sb[:], se1l_p[:])
        se1b_sb = cst.tile([R, 1], f32)
        nc.sync.dma_start(se1b_sb[:], se1b_p[:])
        se2l_sb = cst.tile([61, 2, 120], f32)
        nc.sync.dma_start(se2l_sb[:], se2l_p[:])
        s26_sb = cst.tile([120, 2], f32)
        nc.sync.dma_start(s26_sb[:], s26_p[:])
        pb26_sb = cst.tile([120, 2], f32)
        nc.sync.dma_start(pb26_sb[:], pb26_p[:])
        t2c_sb = cst.tile([120, 2], f32)
        nc.sync.dma_start(t2c_sb[:], t2c_p[:])

        # ActT[ch]: [(w28,b4)=112, q2, h28, c'=128]  (c' = 4*gl + c4; pad 8)
        ActT = [pers.tile([112, 2, H, 128], f16, name=f"actt_{ch}")
                for ch in range(2)]
        # PWrhs[ch]: [c'=128 (120 used), q2, h28, w28, b4]
        PWrhs = [pers.tile([128, 2, H, W, 4], f16, name=f"pwrhs_{ch}")
                 for ch in range(2)]
        # y staging: [o=120, mo2, b8, hw784]
        y_sb = pers.tile([120, 2, NB, HWF], ydt, name="y_sb")
        # SE tensors
        s_sb = [pers.tile([128, NB], f32, name=f"s_{ch}") for ch in range(2)]
        h1 = pers.tile([61, NB], f32, name="h1")
        g_t = [pers.tile([120, NB], f32, name=f"g_{mo}") for mo in range(2)]
        stab = [pers.tile([120, NB], f32, name=f"stab_{mo}")
                for mo in range(2)]
        btab = [pers.tile([120, NB], f32, name=f"btab_{mo}")
                for mo in range(2)]

        # one-time inits: h1 bias row (engine partition base must be
        # 32-aligned, so fill 32:61 with ones; rows 32:60 are overwritten
        # by the SE1 activation before use), ActT pad channel columns
        nc.vector.memset(h1[32:61, :], 1.0)
        for ch in range(2):
            for q in range(2):
                nc.gpsimd.memset(ActT[ch][:, q, :, 120:128], 0.0)

        # ---------------- emission helpers (PE executes in program order,
        # so emission order is the schedule)
        NT = 392  # half an image
        state = {"cpalt": 0, "alt": 0}

        def emit_dw_pair(dwps, mp, ch, gp):
            ps = dwps.tile([112, 2, 2, 112], f32, tag="dw")
            for gi in range(2):
                g = ch * 30 + 2 * gp + gi
                for q in range(2):
                    for dx in range(KK):
                        nc.tensor.matmul(
                            ps[:, gi, q, :],
                            x_sb[:, g, q, dx:dx + 28, :],
                            tc_sb[:, g, dx, :],
                            start=(gi == 0 and q == 0 and dx == 0),
                            stop=(gi == 1 and q == 1 and dx == KK - 1),
                        )
            # hardswish*6: act = (clip(a,-3,3)+3)*a with a = BN1 output.
            # GPSIMD cannot read PSUM, so Act stages psum->SBUF first.
            a3 = mp.tile([112, 2, 2, 112], f32, tag="a3")
            nc.scalar.copy(a3[:], ps[:])
            m = mp.tile([112, 2, 2, 112], f32, tag="m")
            nc.gpsimd.tensor_scalar(m[:], a3[:], 3.0, -3.0, AL.min, AL.max)
            for q in range(2):
                # out free dims (gi, c4, h): (c4,gi) merge to a stride-1 run
                att = ActT[ch][:, q, :, 8 * gp:8 * gp + 8].rearrange(
                    "p h (gi c) -> p gi c h", gi=2)
                nc.vector.scalar_tensor_tensor(
                    att,
                    m[:, :, q, :].rearrange("p gi (c h) -> p gi c h", c=4),
                    3.0,
                    a3[:, :, q, :].rearrange("p gi (c h) -> p gi c h", c=4),
                    AL.add, AL.mult)

        def emit_t2_block(t2pool, ch, q, h0, ssum=None):
            # T2 transposes; the tiny SE-sum matmuls ride along so their
            # sequencer time hides behind the transposes' engine time
            hb = min(8, H - h0)
            tp = t2pool.tile([128, 8, 112], f16, tag="t2")
            for hi in range(hb):
                src = ActT[ch][:, q, h0 + hi, :]
                nc.tensor.transpose(tp[:, hi, :], src, ident[:])
                if ssum is not None:
                    nc.tensor.matmul(
                        ssum[:, 4 * q:4 * q + 4], src, ones4_sb[:],
                        start=(q == 0 and h0 + hi == 0),
                        stop=(q == 1 and h0 + hi == H - 1))
            # copy PSUM->PWrhs split across DVE and Act in parallel
            # (GPSIMD cannot read PSUM)
            cpdst = PWrhs[ch][:, q, h0:h0 + hb].rearrange(
                "p h w b -> p h (w b)")
            hh = 5 if hb == 8 else 3
            nc.vector.tensor_copy(cpdst[:, 0:hh], tp[:, 0:hh, :])
            nc.scalar.copy(cpdst[:, hh:hb], tp[:, hh:hb, :])

        def emit_se_chain(sps, mp):
            ps1 = sps.tile([R, NB], f32, tag="se")
            for ch in range(2):
                nc.tensor.matmul(ps1[:], se1l_sb[:, ch, :],
                                 s_sb[ch][0:120, :],
                                 start=(ch == 0), stop=(ch == 1))
            nc.scalar.activation(h1[0:60, :], ps1[:], AF.Relu,
                                 bias=se1b_sb[:, 0:1])
            for mo in range(2):
                ps2 = sps.tile([120, NB], f32, tag="se")
                nc.tensor.matmul(ps2[:], se2l_sb[:, mo, :], h1[:],
                                 start=True, stop=True)
                m2 = mp.tile([120, NB], f32, tag="m2")
                nc.vector.tensor_scalar(m2[:], ps2[:], 6.0, 0.0,
                                        AL.min, AL.max)
                nc.vector.scalar_tensor_tensor(g_t[mo][:], ps2[:], 3.0,
                                               m2[:], AL.subtract, AL.mult)
                nc.vector.tensor_scalar(stab[mo][:], g_t[mo][:],
                                        s26_sb[:, mo:mo + 1], None, AL.mult)
                nc.vector.tensor_scalar(btab[mo][:], g_t[mo][:],
                                        pb26_sb[:, mo:mo + 1],
                                        t2c_sb[:, mo:mo + 1],
                                        AL.mult, AL.add)

        def emit_pw(pdps, q, b4):
            # complete one whole image (both halves, both output-channel
            # halves) then emit its 2 output DMAs, so y transfers spread
            # across phase D instead of piling up at the end
            b_ = 4 * q + b4
            for nt in range(2):
                for mo in range(2):
                    ps = pdps.tile([120, NT], f32, tag="pw")
                    for kc in range(2):
                        nc.tensor.matmul(
                            ps[:],
                            pwl_sb[:, kc, mo, :],
                            PWrhs[kc][0:120, q, 14 * nt:14 * nt + 14, :, b4],
                            start=(kc == 0), stop=(kc == 1))
                    dst = y_sb[:, mo, b_, NT * nt:NT * nt + NT]
                    if cfg["epi_dve"] and (state["alt"] % 2 == 1):
                        nc.vector.tensor_scalar(
                            dst, ps[:], stab[mo][:, b_:b_ + 1],
                            btab[mo][:, b_:b_ + 1], AL.mult, AL.add)
                    else:
                        nc.scalar.activation(
                            dst, ps[:], AF.Identity,
                            bias=btab[mo][:, b_:b_ + 1],
                            scale=stab[mo][:, b_:b_ + 1])
                    state["alt"] += 1
            for mo in range(2):
                nc.sync.dma_start(
                    y_p[b_:b_ + 1, 120 * mo:120 * mo + 120, :].rearrange(
                        "b c f -> c (b f)"),
                    y_sb[:, mo, b_, :])

        # ============ scope 1: A(ch0), start of A(ch1) to cover the ActT
        # drain, B(ch0), rest of A(ch1), SE sums + chain
        EARLY = cfg["a1_early"]
        with tc.tile_pool(name="dwps", bufs=cfg["dw_bufs"], space="PSUM") \
                as dwps, \
             tc.tile_pool(name="mp", bufs=cfg["m_bufs"]) as mp, \
             tc.tile_pool(name="t2ps", bufs=cfg["t2_bufs"], space="PSUM") \
                as t2ps, \
             tc.tile_pool(name="sps", bufs=1, space="PSUM") as sps:
            for gp in range(15):
                emit_dw_pair(dwps, mp, 0, gp)
            for gp in range(EARLY):
                emit_dw_pair(dwps, mp, 1, gp)
            ssum0 = sps.tile([128, NB], f32, tag="se")
            for q in range(2):
                for h0 in range(0, H, 8):
                    emit_t2_block(t2ps, 0, q, h0, ssum=ssum0)
            nc.vector.tensor_copy(s_sb[0][:], ssum0[:])
            for gp in range(EARLY, 15):
                emit_dw_pair(dwps, mp, 1, gp)

        # ============ scope 2: B(ch1) + SE chain, then PW + epilogue + out
        with tc.tile_pool(name="t2b", bufs=3, space="PSUM") as t2b, \
             tc.tile_pool(name="sps2", bufs=1, space="PSUM") as sps2, \
             tc.tile_pool(name="mp2", bufs=2) as mp2, \
             tc.tile_pool(name="pdps", bufs=cfg["pd_bufs"], space="PSUM") \
                as pdps:
            ssum1 = sps2.tile([128, NB], f32, tag="se")
            for q in range(2):
                for h0 in range(0, H, 8):
                    emit_t2_block(t2b, 1, q, h0, ssum=ssum1)
            nc.vector.tensor_copy(s_sb[1][:], ssum1[:])
            emit_se_chain(sps2, mp2)
            for q in range(2):
                for b4 in range(4):
                    emit_pw(pdps, q, b4)

        pers.release()
        cst.release()

    nc.compile()
    _BUILD_CACHE[key] = nc
    return nc


# ---------------------------------------------------------------- host prep
def prep_inputs(inputs, cfg_key=None):
    f32 = np.float32
    f16 = np.float16

    x = np.asarray(inputs["x"], f32)
    dw_w = np.asarray(inputs["dw_w"], f32)      # [C,1,5,5]
    dw_b = np.asarray(inputs["dw_b"], f32)
    bn1_g = np.asarray(inputs["bn1_g"], f32)
    bn1_b = np.asarray(inputs["bn1_b"], f32)
    bn1_m = np.asarray(inputs["bn1_m"], f32)
    bn1_v = np.asarray(inputs["bn1_v"], f32)
    pw_w = np.asarray(inputs["pw_w"], f32)      # [Cout, C]
    pw_b = np.asarray(inputs["pw_b"], f32)
    se_w1 = np.asarray(inputs["se_w1"], f32)    # [R, C]
    se_b1 = np.asarray(inputs["se_b1"], f32)
    se_w2 = np.asarray(inputs["se_w2"], f32)    # [Cout, R]
    se_b2 = np.asarray(inputs["se_b2"], f32)
    bn2_g = np.asarray(inputs["bn2_g"], f32)
    bn2_b = np.asarray(inputs["bn2_b"], f32)
    bn2_m = np.asarray(inputs["bn2_m"], f32)
    bn2_v = np.asarray(inputs["bn2_v"], f32)

    s1 = bn1_g / np.sqrt(bn1_v + EPS)
    t1 = s1 * (dw_b - bn1_m) + bn1_b

    # expanded block-diag Toeplitz [113=(c4,hin28)+bias, G, KK, 112=(c4,ho)],
    # BN1-scale folded in; row 112 at dx=2 carries the (t1+3) bias.
    hin = np.arange(H)[:, None]
    hout = np.arange(H)[None, :]
    D = hin - hout
    mask = np.abs(D) <= 2
    dyi = np.clip(D + 2, 0, 4)
    kb = dw_w[:, 0] * s1[:, None, None]                    # [C, 5, 5]
    band = np.where(mask[None, :, :, None], kb[:, dyi, :], 0.0)  # [C,hin,ho,dx]
    band_r = band.reshape(G, 4, H, H, KK)                  # [g,ci,hin,ho,dx]
    tcc = np.zeros((113, G, KK, 4, H), f32)                # [row,g,dx,cj,ho]
    for ci in range(4):
        tcc[28 * ci:28 * ci + H, :, :, ci, :] = band_r[:, ci].transpose(
            1, 0, 3, 2)                                    # [hin,g,dx,ho]
    tcc[112, :, 2, :, :] = t1.reshape(G, 4)[:, :, None]
    toepx = tcc.reshape(113, G, KK, 112).astype(f16)

    # pointwise weights [c'=120, ch2, mo2, o=120], /6 hardswish fold
    pwl = np.ascontiguousarray(
        (pw_w.T / 6.0).reshape(2, 120, 2, 120).transpose(1, 0, 2, 3)
    ).astype(f16)

    # SE weights: mean fold = 1/(6*784); c' ordering matches pwl
    se1l = np.ascontiguousarray(
        (se_w1.T / (6.0 * HWF)).reshape(2, 120, R).transpose(1, 0, 2)
    ).astype(f32)
    se1b = se_b1.reshape(R, 1).astype(f32)
    se2l = np.zeros((61, 2, 120), f32)
    se2l[:R] = se_w2.T.reshape(R, 2, 120)
    se2l[60] = (se_b2 + 3.0).reshape(2, 120)

    s2 = bn2_g / np.sqrt(bn2_v + EPS)
    s26 = np.ascontiguousarray((s2 / 6.0).reshape(2, 120).T).astype(f32)
    pb26 = np.ascontiguousarray((pw_b * s2 / 6.0).reshape(2, 120).T).astype(f32)
    t2c = np.ascontiguousarray(
        (bn2_b - bn2_m * s2).reshape(2, 120).T).astype(f32)

    # partition order is (w28, b4): ones4[p, b'] = 1 iff p % 4 == b'
    ones4 = np.kron(np.ones((28, 1), f32), np.eye(4, dtype=f32)).astype(f16)

    shared = {
        "toepx": toepx, "pwl": pwl, "ones4": ones4, "se1l": se1l,
        "se1b": se1b, "se2l": se2l, "s26": s26, "pb26": pb26, "t2c": t2c,
    }
    in_maps = []
    for i in range(N_CORES):
        m = dict(shared)
        # x16 [113=(c4,h28)+ones, G, q2, WP, b4]: w cols 0,1,30,31 zero
        # (row 112 all-ones).
        xc = x[i * NB:(i + 1) * NB]                        # [NB, C, H, W]
        xt = np.zeros((113, G, 2, WP, 4), f32)
        # [c4, h, g, w, q, b4] -> [(c4 h), g, q, w, b4]
        xt[:112, :, :, 2:2 + W, :] = xc.reshape(2, 4, G, 4, H, W).transpose(
            3, 4, 2, 5, 0, 1).reshape(112, G, W, 2, 4).transpose(
            0, 1, 3, 2, 4)
        xt[112] = 1.0
        m["x16"] = np.ascontiguousarray(xt).astype(f16)
        in_maps.append(m)
    return in_maps


def kernel(**inputs):
    nc = build_nc()
    in_maps = prep_inputs(inputs)
    res = run_bass_kernel_spmd(nc, in_maps, list(range(N_CORES)))
    out = np.concatenate(
        [np.asarray(res.results[i]["y16"], dtype=np.float32).reshape(
            NB, C, H, W) for i in range(N_CORES)], axis=0)
    return out


# revision 71
# speedup vs baseline: 2.7278x; 1.0461x over previous
"""Trainium2 Bass kernel for nn_DepthWiseSepConv (depthwise 5x5 + BN+hardswish
+ pointwise 1x1 + squeeze-excite gating + BN), data-parallel over batch on
8 NeuronCores.

Self-contained: hardcodes all shapes from the problem spec.

Per-core design (B_loc = 8 images per core), fp16 matmul path:

  - Depthwise conv as H-Toeplitz matmuls with *x as the stationary operand*:
    per (group-of-4-channels, image-quad q, kernel column dx) one matmul
    with lhsT = x window [113, (w28, b4)=112] (single contiguous free dim)
    and rhs = expanded block-diagonal Toeplitz [113, (c4, hout)=112],
    accumulating the 5 dx in PSUM.  The output lands (w,b)-major, i.e.
    already transposed, eliminating one transpose pass entirely.
  - BN1 is folded into the matmul: Toeplitz values are pre-scaled by the
    BN scale, row 112 of x is constant 1.0, and row 112 of the Toeplitz
    (at dx=2) carries the BN shift t1, so PSUM holds the BN1 output.
  - hardswish*6: act = (clip(a,-3,3)+3)*a split as Act psum->SBUF copy,
    Pool clip, DVE combine (GPSIMD cannot read PSUM); the /6 is folded
    into the pointwise and SE weights host-side.
  - T2: PE transposes ActT[(w,b), (q,h,c')] slices into channel-major
    PWrhs[c', (q,h,w,b)]; PSUM->SBUF copies split Act/DVE.  SE channel
    sums are tiny ones-matmuls on PE (exact f32, nearly free).
  - SE: two small fp32 matmuls; the SE2 bias rides a constant-1 row of h1.
    The gate folds into per-(channel,batch) scale/bias tables so the whole
    PW epilogue (pw_b, gate, BN2) is a single op per tile.
  - Pointwise conv: [120x120] fp16 matmuls over half-images (N=392, rhs
    strided over the (w,b)-interleaved pixels), epilogue alternates
    Act/DVE, output staged fp16 and upcast to f32 on host.
  - Emission order is tuned so phase A overlaps the (DMA-paced) x/Toeplitz
    load stream, SE sums close early, and PW interleaves with the second
    transpose phase.
"""

import sys

sys.path.insert(0, "/opt/trn_rl_repo")

import numpy as np

import concourse.mybir as mybir
import concourse.tile as tile
from concourse import bacc
from concourse.bass_utils import run_bass_kernel_spmd
from concourse.masks import make_identity

# ---------------------------------------------------------------- constants
N_CORES = 8
B, C, H, W = 64, 240, 28, 28
NB = B // N_CORES           # images per core
KK = 5                      # depthwise kernel size
G = C // 4                  # channel groups of 4 -> 60
R = 60                      # SE reduction dim
HWF = H * W                 # 784
EPS = 1e-5
WP = 32                     # padded W in SBUF (w in [-2, 30))

CFG = {
    "ld_step": 4,           # groups per x/toeplitz load DMA
    "ld_head": (2, 2),      # leading chunk sizes
    "dw_bufs": 4,           # DW psum pool depth (2 groups per bank)
    "m_bufs": 4,            # hardswish temp pool depth
    "t2_bufs": 3,           # T2 psum pool depth
    "pd_bufs": 4,           # PW psum pool depth
    "a1_early": 15,         # A(ch1) pairs emitted before B(ch0)
    "epi_dve": True,        # alternate PW epilogue between Act and DVE
    "y_f32": False,         # emit f32 output instead of fp16+host upcast
}

_DT16 = mybir.dt.float16
_F32 = mybir.dt.float32


# ---------------------------------------------------------------- builder
_BUILD_CACHE = {}


def build_nc(cfg_key=None):
    cfg = dict(CFG)
    if cfg_key is not None:
        cfg.update(cfg_key)
    cfg["ld_head"] = tuple(cfg["ld_head"])
    key = tuple(sorted(cfg.items()))
    if key in _BUILD_CACHE:
        return _BUILD_CACHE[key]

    f16, f32 = _DT16, _F32
    AL = mybir.AluOpType
    AF = mybir.ActivationFunctionType
    ydt = f32 if cfg["y_f32"] else f16

    nc = bacc.Bacc("TRN2", target_bir_lowering=False, debug=False,
                   num_devices=N_CORES)

    # x layout [113, G, q2, w32, b4]: (w, b)-inner so a 28-w window over
    # 4 images merges to ONE contiguous free dim (matmul weights APs must
    # have a single free dimension)
    x_p = nc.declare_dram_parameter("x16", [113, G, 2, WP, 4], f16,
                                    isOutput=False)
    tc_p = nc.declare_dram_parameter("toepx", [113, G, KK, 112], f16,
                                     isOutput=False)
    pwl_p = nc.declare_dram_parameter("pwl", [120, 2, 2, 120], f16,
                                      isOutput=False)
    ones4_p = nc.declare_dram_parameter("ones4", [112, 4], f16,
                                        isOutput=False)
    se1l_p = nc.declare_dram_parameter("se1l", [120, 2, R], f32,
                                       isOutput=False)
    se1b_p = nc.declare_dram_parameter("se1b", [R, 1], f32, isOutput=False)
    se2l_p = nc.declare_dram_parameter("se2l", [61, 2, 120], f32,
                                       isOutput=False)
    s26_p = nc.declare_dram_parameter("s26", [120, 2], f32, isOutput=False)
    pb26_p = nc.declare_dram_parameter("pb26", [120, 2], f32, isOutput=False)
    t2c_p = nc.declare_dram_parameter("t2c", [120, 2], f32, isOutput=False)
    y_p = nc.declare_dram_parameter("y16", [NB, C, HWF], ydt, isOutput=True)

    with tile.TileContext(nc) as tc:
        cst = tc.alloc_tile_pool(name="cst", bufs=1)
        pers = tc.alloc_tile_pool(name="pers", bufs=1)

        # ---- x / toeplitz first, interleaved by group range, so phase A can
        # start as soon as the first chunk lands
        x_sb = pers.tile([113, G, 2, WP, 4], f16, name="x16")
        tc_sb = pers.tile([113, G, KK, 112], f16, name="toepx")
        step = cfg["ld_step"]
        sizes = list(cfg["ld_head"])
        while sum(sizes) < G:
            sizes.append(min(step, G - sum(sizes)))
        a = 0
        for sz in sizes:
            b_ = a + sz
            nc.sync.dma_start(tc_sb[:, a:b_], tc_p[:, a:b_])
            nc.sync.dma_start(x_sb[:, a:b_], x_p[:, a:b_])
            a = b_

        # ---- constants in SBUF
        ident = cst.tile([112, 112], f16)
        make_identity(nc, ident[:])
        ones4_sb = cst.tile([112, 4], f16)
        nc.sync.dma_start(ones4_sb[:], ones4_p[:])
        pwl_sb = cst.tile([120, 2, 2, 120], f16)
        nc.sync.dma_start(pwl_sb[:], pwl_p[:])
        se1l_sb = cst.tile([120, 2, R], f32)
        nc.sync.dma_start(se1l_sb[:], se1l_p[:])
        se1b_sb = cst.tile([R, 1], f32)
        nc.sync.dma_start(se1b_sb[:], se1b_p[:])
        se2l_sb = cst.tile([61, 2, 120], f32)
        nc.sync.dma_start(se2l_sb[:], se2l_p[:])
        s26_sb = cst.tile([120, 2], f32)
        nc.sync.dma_start(s26_sb[:], s26_p[:])
        pb26_sb = cst.tile([120, 2], f32)
        nc.sync.dma_start(pb26_sb[:], pb26_p[:])
        t2c_sb = cst.tile([120, 2], f32)
        nc.sync.dma_start(t2c_sb[:], t2c_p[:])

        # ActT[ch]: [(w28,b4)=112, q2, h28, c'=128]  (c' = 4*gl + c4; pad 8)
        ActT = [pers.tile([112, 2, H, 128], f16, name=f"actt_{ch}")
                for ch in range(2)]
        # PWrhs[ch]: [c'=128 (120 used), q2, h28, w28, b4]
        PWrhs = [pers.tile([128, 2, H, W, 4], f16, name=f"pwrhs_{ch}")
                 for ch in range(2)]
        # y staging: [o=120, mo2, b8, hw784]
        y_sb = pers.tile([120, 2, NB, HWF], ydt, name="y_sb")
        # SE tensors
        s_sb = [pers.tile([128, NB], f32, name=f"s_{ch}") for ch in range(2)]
        h1 = pers.tile([61, NB], f32, name="h1")
        g_t = [pers.tile([120, NB], f32, name=f"g_{mo}") for mo in range(2)]
        stab = [pers.tile([120, NB], f32, name=f"stab_{mo}")
                for mo in range(2)]
        btab = [pers.tile([120, NB], f32, name=f"btab_{mo}")
                for mo in range(2)]

        # one-time inits: h1 bias row (engine partition base must be
        # 32-aligned, so fill 32:61 with ones; rows 32:60 are overwritten
        # by the SE1 activation before use), ActT pad channel columns
        nc.vector.memset(h1[32:61, :], 1.0)
        for ch in range(2):
            for q in range(2):
                nc.gpsimd.memset(ActT[ch][:, q, :, 120:128], 0.0)

        # ---------------- emission helpers (PE executes in program order,
        # so emission order is the schedule)
        NT = 392  # half an image
        state = {"cpalt": 0, "alt": 0}

        def emit_dw(dwps, mp, ch, g0, ng):
            # ng groups (1 or 2) share one psum bank
            ps = dwps.tile([112, 2, 2, 112], f32, tag="dw")
            for gi in range(ng):
                g = ch * 30 + g0 + gi
                for q in range(2):
                    for dx in range(KK):
                        nc.tensor.matmul(
                            ps[:, gi, q, :],
                            x_sb[:, g, q, dx:dx + 28, :],
                            tc_sb[:, g, dx, :],
                            start=(gi == 0 and q == 0 and dx == 0),
                            stop=(gi == ng - 1 and q == 1 and dx == KK - 1),
                        )
            # hardswish*6: act = (clip(a,-3,3)+3)*a with a = BN1 output.
            # GPSIMD cannot read PSUM, so Act stages psum->SBUF first.
            a3 = mp.tile([112, 2, 2, 112], f32, tag="a3")
            nc.scalar.copy(a3[:, 0:ng], ps[:, 0:ng])
            m = mp.tile([112, 2, 2, 112], f32, tag="m")
            nc.gpsimd.tensor_scalar(m[:, 0:ng], a3[:, 0:ng], 3.0, -3.0,
                                    AL.min, AL.max)
            for q in range(2):
                # out free dims (gi, c4, h): (c4,gi) merge to a stride-1 run
                att = ActT[ch][:, q, :, 4 * g0:4 * g0 + 4 * ng].rearrange(
                    "p h (gi c) -> p gi c h", gi=ng)
                nc.vector.scalar_tensor_tensor(
                    att,
                    m[:, 0:ng, q, :].rearrange("p gi (c h) -> p gi c h", c=4),
                    3.0,
                    a3[:, 0:ng, q, :].rearrange("p gi (c h) -> p gi c h", c=4),
                    AL.add, AL.mult)

        def emit_ssum_chunk(ssum, ch, q, h0, hn):
            # tiny SE-sum matmuls (engine ~2ns, SEQ ~25ns each): emit in
            # chunks where the PE sequencer has slack
            for h in range(h0, h0 + hn):
                nc.tensor.matmul(
                    ssum[:, 4 * q:4 * q + 4], ActT[ch][:, q, h, :],
                    ones4_sb[:],
                    start=(q == 0 and h == 0),
                    stop=(q == 1 and h == H - 1))

        def emit_t2_block(t2pool, ch, q, h0):
            hb = min(8, H - h0)
            tp = t2pool.tile([128, 8, 112], f16, tag="t2")
            for hi in range(hb):
                src = ActT[ch][:, q, h0 + hi, :]
                nc.tensor.transpose(tp[:, hi, :], src, ident[:])
            # copy PSUM->PWrhs split across DVE and Act in parallel
            # (GPSIMD cannot read PSUM)
            cpdst = PWrhs[ch][:, q, h0:h0 + hb].rearrange(
                "p h w b -> p h (w b)")
            hh = 5 if hb == 8 else 3
            nc.vector.tensor_copy(cpdst[:, 0:hh], tp[:, 0:hh, :])
            nc.scalar.copy(cpdst[:, hh:hb], tp[:, hh:hb, :])

        def emit_se_chain(sps, mp):
            ps1 = sps.tile([R, NB], f32, tag="se")
            for ch in range(2):
                nc.tensor.matmul(ps1[:], se1l_sb[:, ch, :],
                                 s_sb[ch][0:120, :],
                                 start=(ch == 0), stop=(ch == 1))
            nc.scalar.activation(h1[0:60, :], ps1[:], AF.Relu,
                                 bias=se1b_sb[:, 0:1])
            for mo in range(2):
                ps2 = sps.tile([120, NB], f32, tag="se")
                nc.tensor.matmul(ps2[:], se2l_sb[:, mo, :], h1[:],
                                 start=True, stop=True)
                m2 = mp.tile([120, NB], f32, tag="m2")
                nc.vector.tensor_scalar(m2[:], ps2[:], 6.0, 0.0,
                                        AL.min, AL.max)
                nc.vector.scalar_tensor_tensor(g_t[mo][:], ps2[:], 3.0,
                                               m2[:], AL.subtract, AL.mult)
                nc.vector.tensor_scalar(stab[mo][:], g_t[mo][:],
                                        s26_sb[:, mo:mo + 1], None, AL.mult)
                nc.vector.tensor_scalar(btab[mo][:], g_t[mo][:],
                                        pb26_sb[:, mo:mo + 1],
                                        t2c_sb[:, mo:mo + 1],
                                        AL.mult, AL.add)

        def emit_pw(pdps, q, b4):
            # complete one whole image (both halves, both output-channel
            # halves) then emit its 2 output DMAs, so y transfers spread
            # across phase D instead of piling up at the end
            b_ = 4 * q + b4
            for mo in range(2):
                for nt in range(2):
                    ps = pdps.tile([120, NT], f32, tag="pw")
                    for kc in range(2):
                        nc.tensor.matmul(
                            ps[:],
                            pwl_sb[:, kc, mo, :],
                            PWrhs[kc][0:120, q, 14 * nt:14 * nt + 14, :, b4],
                            start=(kc == 0), stop=(kc == 1))
                    dst = y_sb[:, mo, b_, NT * nt:NT * nt + NT]
                    if cfg["epi_dve"] and (state["alt"] % 2 == 1):
                        nc.vector.tensor_scalar(
                            dst, ps[:], stab[mo][:, b_:b_ + 1],
                            btab[mo][:, b_:b_ + 1], AL.mult, AL.add)
                    else:
                        nc.scalar.activation(
                            dst, ps[:], AF.Identity,
                            bias=btab[mo][:, b_:b_ + 1],
                            scale=stab[mo][:, b_:b_ + 1])
                    state["alt"] += 1
                # y DMA for (mo, image) right after its two tiles, so the
                # mo=0 transfer overlaps mo=1 compute
                nc.sync.dma_start(
                    y_p[b_:b_ + 1, 120 * mo:120 * mo + 120, :].rearrange(
                        "b c f -> c (b f)"),
                    y_sb[:, mo, b_, :])

        # ============ scope 1: A(ch0), start of A(ch1) to cover the ActT
        # drain, B(ch0), rest of A(ch1), SE sums + chain
        EARLY = cfg["a1_early"]
        with tc.tile_pool(name="dwps", bufs=cfg["dw_bufs"], space="PSUM") \
                as dwps, \
             tc.tile_pool(name="mp", bufs=cfg["m_bufs"]) as mp, \
             tc.tile_pool(name="t2ps", bufs=cfg["t2_bufs"], space="PSUM") \
                as t2ps, \
             tc.tile_pool(name="sps", bufs=1, space="PSUM") as sps:
            for gp in range(15):
                emit_dw(dwps, mp, 0, 2 * gp, 2)
            for gp in range(min(EARLY, 14)):
                emit_dw(dwps, mp, 1, 2 * gp, 2)
            ssum0 = sps.tile([128, NB], f32, tag="se")
            # ssum0's SEQ-heavy matmuls ride between A(ch1) DW pairs
            chunks = [(q, h0) for q in range(2) for h0 in range(0, H, 7)]
            for gp in range(min(EARLY, 14), 14):
                emit_dw(dwps, mp, 1, 2 * gp, 2)
                if chunks and gp >= 4:
                    q, h0 = chunks.pop(0)
                    emit_ssum_chunk(ssum0, 0, q, h0, 7)
            # last two groups as singles: halves the final ActT drain that
            # gates phase B(ch1)
            emit_dw(dwps, mp, 1, 28, 1)
            emit_dw(dwps, mp, 1, 29, 1)
            for q in range(2):
                for h0 in range(0, H, 8):
                    emit_t2_block(t2ps, 0, q, h0)
            for q, h0 in chunks:
                emit_ssum_chunk(ssum0, 0, q, h0, 7)
            nc.vector.tensor_copy(s_sb[0][:], ssum0[:])

        # ============ scope 2: B(ch1) + SE chain, then PW + epilogue + out
        with tc.tile_pool(name="t2b", bufs=3, space="PSUM") as t2b, \
             tc.tile_pool(name="sps2", bufs=1, space="PSUM") as sps2, \
             tc.tile_pool(name="mp2", bufs=2) as mp2, \
             tc.tile_pool(name="pdps", bufs=cfg["pd_bufs"], space="PSUM") \
                as pdps:
            # dense SE-sum first so its accumulation closes (and the
            # epilogue tables become ready) before the T2/PW stream drains
            ssum1 = sps2.tile([128, NB], f32, tag="se")
            for q in range(2):
                emit_ssum_chunk(ssum1, 1, q, 0, H)
            nc.vector.tensor_copy(s_sb[1][:], ssum1[:])
            emit_se_chain(sps2, mp2)
            for q in range(2):
                for h0 in range(0, H, 8):
                    emit_t2_block(t2b, 1, q, h0)
                if q == 0:
                    emit_pw(pdps, 0, 0)
                    emit_pw(pdps, 0, 1)
            for b4 in range(2, 4):
                emit_pw(pdps, 0, b4)
            for b4 in range(4):
                emit_pw(pdps, 1, b4)

        pers.release()
        cst.release()

    nc.compile()
    _BUILD_CACHE[key] = nc
    return nc


# ---------------------------------------------------------------- host prep
def prep_inputs(inputs, cfg_key=None):
    f32 = np.float32
    f16 = np.float16

    x = np.asarray(inputs["x"], f32)
    dw_w = np.asarray(inputs["dw_w"], f32)      # [C,1,5,5]
    dw_b = np.asarray(inputs["dw_b"], f32)
    bn1_g = np.asarray(inputs["bn1_g"], f32)
    bn1_b = np.asarray(inputs["bn1_b"], f32)
    bn1_m = np.asarray(inputs["bn1_m"], f32)
    bn1_v = np.asarray(inputs["bn1_v"], f32)
    pw_w = np.asarray(inputs["pw_w"], f32)      # [Cout, C]
    pw_b = np.asarray(inputs["pw_b"], f32)
    se_w1 = np.asarray(inputs["se_w1"], f32)    # [R, C]
    se_b1 = np.asarray(inputs["se_b1"], f32)
    se_w2 = np.asarray(inputs["se_w2"], f32)    # [Cout, R]
    se_b2 = np.asarray(inputs["se_b2"], f32)
    bn2_g = np.asarray(inputs["bn2_g"], f32)
    bn2_b = np.asarray(inputs["bn2_b"], f32)
    bn2_m = np.asarray(inputs["bn2_m"], f32)
    bn2_v = np.asarray(inputs["bn2_v"], f32)

    s1 = bn1_g / np.sqrt(bn1_v + EPS)
    t1 = s1 * (dw_b - bn1_m) + bn1_b

    # expanded block-diag Toeplitz [113=(c4,hin28)+bias, G, KK, 112=(c4,ho)],
    # BN1-scale folded in; row 112 at dx=2 carries the (t1+3) bias.
    hin = np.arange(H)[:, None]
    hout = np.arange(H)[None, :]
    D = hin - hout
    mask = np.abs(D) <= 2
    dyi = np.clip(D + 2, 0, 4)
    kb = dw_w[:, 0] * s1[:, None, None]                    # [C, 5, 5]
    band = np.where(mask[None, :, :, None], kb[:, dyi, :], 0.0)  # [C,hin,ho,dx]
    band_r = band.reshape(G, 4, H, H, KK)                  # [g,ci,hin,ho,dx]
    tcc = np.zeros((113, G, KK, 4, H), f32)                # [row,g,dx,cj,ho]
    for ci in range(4):
        tcc[28 * ci:28 * ci + H, :, :, ci, :] = band_r[:, ci].transpose(
            1, 0, 3, 2)                                    # [hin,g,dx,ho]
    tcc[112, :, 2, :, :] = t1.reshape(G, 4)[:, :, None]
    toepx = tcc.reshape(113, G, KK, 112).astype(f16)

    # pointwise weights [c'=120, ch2, mo2, o=120], /6 hardswish fold
    pwl = np.ascontiguousarray(
        (pw_w.T / 6.0).reshape(2, 120, 2, 120).transpose(1, 0, 2, 3)
    ).astype(f16)

    # SE weights: mean fold = 1/(6*784); c' ordering matches pwl
    se1l = np.ascontiguousarray(
        (se_w1.T / (6.0 * HWF)).reshape(2, 120, R).transpose(1, 0, 2)
    ).astype(f32)
    se1b = se_b1.reshape(R, 1).astype(f32)
    se2l = np.zeros((61, 2, 120), f32)
    se2l[:R] = se_w2.T.reshape(R, 2, 120)
    se2l[60] = (se_b2 + 3.0).reshape(2, 120)

    s2 = bn2_g / np.sqrt(bn2_v + EPS)
    s26 = np.ascontiguousarray((s2 / 6.0).reshape(2, 120).T).astype(f32)
    pb26 = np.ascontiguousarray((pw_b * s2 / 6.0).reshape(2, 120).T).astype(f32)
    t2c = np.ascontiguousarray(
        (bn2_b - bn2_m * s2).reshape(2, 120).T).astype(f32)

    # partition order is (w28, b4): ones4[p, b'] = 1 iff p % 4 == b'
    ones4 = np.kron(np.ones((28, 1), f32), np.eye(4, dtype=f32)).astype(f16)

    shared = {
        "toepx": toepx, "pwl": pwl, "ones4": ones4, "se1l": se1l,
        "se1b": se1b, "se2l": se2l, "s26": s26, "pb26": pb26, "t2c": t2c,
    }
    in_maps = []
    for i in range(N_CORES):
        m = dict(shared)
        # x16 [113=(c4,h28)+ones, G, q2, WP, b4]: w cols 0,1,30,31 zero
        # (row 112 all-ones).
        xc = x[i * NB:(i + 1) * NB]                        # [NB, C, H, W]
        xt = np.zeros((113, G, 2, WP, 4), f32)
        # [c4, h, g, w, q, b4] -> [(c4 h), g, q, w, b4]
        xt[:112, :, :, 2:2 + W, :] = xc.reshape(2, 4, G, 4, H, W).transpose(
            3, 4, 2, 5, 0, 1).reshape(112, G, W, 2, 4).transpose(
            0, 1, 3, 2, 4)
        xt[112] = 1.0
        m["x16"] = np.ascontiguousarray(xt).astype(f16)
        in_maps.append(m)
    return in_maps


def kernel(**inputs):
    nc = build_nc()
    in_maps = prep_inputs(inputs)
    res = run_bass_kernel_spmd(nc, in_maps, list(range(N_CORES)))
    out = np.concatenate(
        [np.asarray(res.results[i]["y16"], dtype=np.float32).reshape(
            NB, C, H, W) for i in range(N_CORES)], axis=0)
    return out
